# revision 14
# baseline (speedup 1.0000x reference)
"""Distributed causal multi-head attention for TRN2 (8 NeuronCores).

Problem: x[4,2048,1024] -> qkv proj (c_attn) -> 16-head causal attention
         -> output proj (c_proj).  N_HEAD=16, hd=64.

Sharding (zero collectives): core c handles batch b=c//2 and head-group
hg=c%2 (8 heads).  Each core computes q^T,k^T (transposed layout),
v (natural layout), causal attention with scores computed transposed
(S^T[j,i] = k_j . q_i), softmax without max-subtraction (inputs bounded;
masked future tiles skipped entirely, diagonal tiles masked by a post-exp
0/1 multiply), PV via an appended ones-column in V giving row sums for
free, then the c_proj partial product in transposed layout out^T[e,s].
Host sums the two head-group partials per batch, transposes, adds bias.
"""

import sys

if "/opt/trn_rl_repo" not in sys.path:
    sys.path.insert(0, "/opt/trn_rl_repo")

import numpy as np
import ml_dtypes

import concourse.bass as bass
import concourse.mybir as mybir
import concourse.tile as tile
from concourse import bacc
from concourse.bass_utils import run_bass_kernel_spmd

BF16 = mybir.dt.bfloat16
F32 = mybir.dt.float32

S = 2048            # sequence length
D = 1024            # model dim
H = 16              # total heads
HLOC = 8            # heads per core
HD = 64             # head dim
DQK = HLOC * HD     # 512: per-core q (or k) width
NDC = D // 128      # 8 d-chunks (contraction for qkv)
NSS = S // 512      # 4 i-supertiles
NST = S // 128      # 16 s-tiles / j-chunks
NRC = DQK // 128    # 4 contraction chunks for proj

_CACHED = None


def _build():
    nc = bacc.Bacc()

    xt_e = nc.declare_dram_parameter("xt", [D, S], BF16, isOutput=False)
    wqk_e = nc.declare_dram_parameter("wqk", [D, 2 * DQK], BF16, isOutput=False)
    wv_e = nc.declare_dram_parameter("wv", [D, DQK], BF16, isOutput=False)
    wp_e = nc.declare_dram_parameter("wp", [DQK, D], BF16, isOutput=False)
    bqk_e = nc.declare_dram_parameter("bqk", [128, 8], F32, isOutput=False)
    bv_e = nc.declare_dram_parameter("bv", [HD, HLOC], F32, isOutput=False)
    # wide diagonal masks: g=0 covers j-chunk offsets 0,1; g=1 covers 2,3
    msk_e = nc.declare_dram_parameter("msk", [2, 128, 1024], BF16, isOutput=False)
    out_e = nc.declare_dram_parameter("outT", [D, S], F32, isOutput=True)

    with tile.TileContext(nc) as tc:
        with tc.tile_pool(name="res", bufs=1) as res:
            # resident SBUF tensors
            xt = [res.tile([128, S], BF16, tag=f"xt{i}", name=f"xt{i}") for i in range(NDC)]
            wqk = [res.tile([128, 2 * DQK], BF16, tag=f"wqk{i}", name=f"wqk{i}") for i in range(NDC)]
            wv = [res.tile([128, DQK], BF16, tag=f"wv{i}", name=f"wv{i}") for i in range(NDC)]
            wp = [res.tile([128, D], BF16, tag=f"wp{i}", name=f"wp{i}") for i in range(NRC)]
            bqk = res.tile([128, 8], F32, tag="bqk", name="bqk_sb")
            bv = res.tile([HD, HLOC], F32, tag="bv", name="bv_sb")
            msk = [res.tile([128, 1024], BF16, tag=f"msk{i}", name=f"msk{i}") for i in range(2)]
            # q^T,k^T resident: tiles 0..3 = q e-blocks, 4..7 = k e-blocks
            qkT = [res.tile([128, S], BF16, tag=f"qkT{i}", name=f"qkT{i}") for i in range(8)]
            # v in natural layout, augmented with a ones column per head:
            # tile st: [128 j, 8*65] with cols h*65..h*65+63 = v_h, h*65+64 = 1
            vA = [res.tile([128, HLOC * (HD + 1)], BF16, tag=f"v{i}", name=f"v{i}")
                  for i in range(NST)]
            # normalized a^T (heads packed in pairs: head h -> tile h//2,
            # partitions (h%2)*64..)
            aT = [res.tile([128, S], BF16, tag=f"aT{i}", name=f"aT{i}") for i in range(NRC)]

            for i in range(NDC):
                nc.sync.dma_start(out=xt[i][:], in_=xt_e[i * 128:(i + 1) * 128, :])
                nc.sync.dma_start(out=wqk[i][:], in_=wqk_e[i * 128:(i + 1) * 128, :])
                nc.sync.dma_start(out=wv[i][:], in_=wv_e[i * 128:(i + 1) * 128, :])
            for i in range(NRC):
                nc.sync.dma_start(out=wp[i][:], in_=wp_e[i * 128:(i + 1) * 128, :])
            nc.sync.dma_start(out=bqk[:], in_=bqk_e[:])
            nc.sync.dma_start(out=bv[:], in_=bv_e[:])
            for g in range(2):
                nc.sync.dma_start(out=msk[g][:], in_=msk_e[g])
            for st in range(NST):
                va = vA[st]
                nc.vector.memset(
                    va.rearrange("p (h c) -> p h c", c=HD + 1)[:, :, HD:HD + 1], 1.0)

            # ---- phase B1: q^T, k^T  (out^T form: lhsT = w block, rhs = x^T)
            with tc.tile_pool(name="psB", bufs=8, space="PSUM") as psB:
                # emit q/k e-blocks interleaved so attention can start early
                for eb in (0, 4, 1, 5, 2, 6, 3, 7):
                    pq = [psB.tile([128, 512], F32, tag="psB", name="pq") for _ in range(NSS)]
                    for dc in range(NDC):
                        for ss in range(NSS):
                            nc.tensor.matmul(
                                pq[ss][:],
                                wqk[dc][:, eb * 128:(eb + 1) * 128],
                                xt[dc][:, ss * 512:(ss + 1) * 512],
                                start=(dc == 0), stop=(dc == NDC - 1))
                    for ss in range(NSS):
                        nc.vector.tensor_scalar_add(
                            qkT[eb][:, ss * 512:(ss + 1) * 512],
                            pq[ss][:], bqk[:, eb:eb + 1])

            # ---- phase B2: v natural (lhsT = x^T block, rhs = w_v)
            with tc.tile_pool(name="psV", bufs=4, space="PSUM") as psV:
                for st in range(NST):
                    pv = psV.tile([128, DQK], F32, tag="psV", name="pv")
                    for dc in range(NDC):
                        nc.tensor.matmul(
                            pv[:],
                            xt[dc][:, st * 128:(st + 1) * 128],
                            wv[dc][:],
                            start=(dc == 0), stop=(dc == NDC - 1))
                    nc.vector.tensor_copy(
                        vA[st].rearrange("p (h c) -> p h c", c=HD + 1)[:, :, 0:HD],
                        pv.rearrange("p (h c) -> p h c", c=HD))

            # ---- phase C: attention, head by head
            with tc.tile_pool(name="psS", bufs=2, space="PSUM") as psS, \
                 tc.tile_pool(name="psA", bufs=2, space="PSUM") as psA, \
                 tc.tile_pool(name="att", bufs=3) as att, \
                 tc.tile_pool(name="attf", bufs=2) as attf, \
                 tc.tile_pool(name="dscr", bufs=4, space="DRAM") as dscr:
                for h in range(HLOC):
                    qt = qkT[h // 2]          # q^T e-block tile for this head
                    kt = qkT[4 + h // 2]      # k^T e-block tile
                    po = (h % 2) * 64         # partition offset within tile
                    for ss in range(NSS):
                        njc = 4 * ss + 4      # causal: j-chunks 0..4ss+3
                        pa = psA.tile([HD + 1, 512], F32, tag="psA", name="pa")
                        for jg in range(njc // 2):
                            ps = psS.tile([128, 1024], F32, tag="psS", name="ps")
                            for u in range(2):
                                jc = 2 * jg + u
                                nc.tensor.matmul(
                                    ps[:, u * 512:(u + 1) * 512],
                                    kt[po:po + HD, jc * 128:(jc + 1) * 128],
                                    qt[po:po + HD, ss * 512:(ss + 1) * 512],
                                    start=True, stop=True)
                            pt = att.tile([128, 1024], BF16, tag="pt", name="pt")
                            nc.scalar.activation(
                                pt[:], ps[:],
                                mybir.ActivationFunctionType.Exp, scale=0.125)
                            if 2 * jg >= 4 * ss:  # diagonal pair -> mask
                                nc.vector.tensor_mul(
                                    pt[:], pt[:], msk[jg - 2 * ss][:])
                            for u in range(2):
                                jc = 2 * jg + u
                                nc.tensor.matmul(
                                    pa[:],
                                    vA[jc][:, h * (HD + 1):(h + 1) * (HD + 1)],
                                    pt[:, u * 512:(u + 1) * 512],
                                    start=(jc == 0), stop=(jc == njc - 1))
                        # epilogue: sums -> 1/sum -> broadcast -> normalize+bias
                        inv = attf.tile([128, 512], F32, tag="inv", name="inv")
                        nc.vector.reciprocal(inv[64:65, :], pa[HD:HD + 1, :])
                        scr = dscr.tile([512], F32, tag="scr", name="scr")
                        nc.sync.dma_start(out=scr[:], in_=inv[64:65, :])
                        bc = attf.tile([64, 512], F32, tag="bc", name="bc")
                        nc.sync.dma_start(
                            out=bc[:],
                            in_=bass.AP(tensor=scr.tensor, offset=scr.offset,
                                        ap=[[0, 64]] + list(scr.ap)))
                        st1 = attf.tile([64, 512], F32, tag="st1", name="st1")
                        nc.vector.tensor_mul(st1[:], pa[0:HD, :], bc[:])
                        st2 = attf.tile([64, 512], BF16, tag="st2", name="st2")
                        nc.vector.tensor_scalar_add(
                            st2[:], st1[:], bv[:, h:h + 1])
                        nc.sync.dma_start(
                            out=aT[h // 2][po:po + HD, ss * 512:(ss + 1) * 512],
                            in_=st2[:])

            # ---- phase D: out^T = wp^T-block-stationary @ a^T
            with tc.tile_pool(name="psO", bufs=8, space="PSUM") as psO, \
                 tc.tile_pool(name="osb", bufs=6) as osb:
                for eb in range(8):
                    pouts = [psO.tile([128, 512], F32, tag="psO", name="po")
                             for _ in range(NSS)]
                    for rc in range(NRC):
                        for ss in range(NSS):
                            nc.tensor.matmul(
                                pouts[ss][:],
                                wp[rc][:, eb * 128:(eb + 1) * 128],
                                aT[rc][:, ss * 512:(ss + 1) * 512],
                                start=(rc == 0), stop=(rc == NRC - 1))
                    for ss in range(NSS):
                        ot = osb.tile([128, 512], F32, tag="ot", name="ot")
                        nc.vector.tensor_copy(ot[:], pouts[ss][:])
                        nc.sync.dma_start(
                            out=out_e[eb * 128:(eb + 1) * 128,
                                      ss * 512:(ss + 1) * 512],
                            in_=ot[:])

    nc.finalize()
    return nc


def get_graph():
    global _CACHED
    if _CACHED is None:
        _CACHED = _build()
    return _CACHED


def _make_masks():
    """Masks for the wide exp tiles.

    A pt tile at (head, ss, jg) is [128, 1024]: half u (i columns
    u*512..u*512+511 of the tile) holds S^T for j-chunk jc=2*jg+u over the
    i-window [ss*512, ss*512+512).  Diagonal groups are jg with
    2*jg >= 4*ss, i.e. relative offsets (rr = jc - 4*ss) pairs (0,1), (2,3).
    Valid element: global j <= global i:
      jj + rr*128 <= ii  (ii in [0,512) relative to the i-super).
    Group g mask tile [128, 1024]: half u uses rr = 2*g + u.
    """
    out = np.zeros((2, 128, 1024), dtype=ml_dtypes.bfloat16)
    jj = np.arange(128)[:, None]
    ii = np.arange(512)[None, :]
    for g in range(2):
        for u in range(2):
            rr = 2 * g + u
            out[g, :, u * 512:(u + 1) * 512] = (jj + rr * 128 <= ii).astype(
                ml_dtypes.bfloat16)
    return out


def _shard_inputs(x, c_attn_w, c_attn_b, c_proj_w, c_proj_b):
    bf = ml_dtypes.bfloat16
    msk = _make_masks()
    in_maps = []
    for c in range(8):
        b, hg = c // 2, c % 2
        qcols = slice(hg * DQK, hg * DQK + DQK)
        kcols = slice(D + hg * DQK, D + hg * DQK + DQK)
        vcols = slice(2 * D + hg * DQK, 2 * D + hg * DQK + DQK)
        wqk = np.concatenate(
            [c_attn_w[:, qcols], c_attn_w[:, kcols]], axis=1).astype(bf)
        wv = np.ascontiguousarray(c_attn_w[:, vcols]).astype(bf)
        wp = np.ascontiguousarray(
            c_proj_w[hg * DQK:hg * DQK + DQK, :]).astype(bf)
        bq = np.concatenate(
            [c_attn_b[qcols], c_attn_b[kcols]]).astype(np.float32)
        bqk = np.ascontiguousarray(bq.reshape(8, 128).T)  # [128, 8]
        bvv = np.ascontiguousarray(
            c_attn_b[vcols].astype(np.float32).reshape(HLOC, HD).T)  # [64, 8]
        xt = np.ascontiguousarray(x[b].T).astype(bf)
        in_maps.append({
            "xt": xt, "wqk": wqk, "wv": wv, "wp": wp,
            "bqk": bqk, "bv": bvv, "msk": msk,
        })
    return in_maps


def kernel(x, c_attn_w, c_attn_b, c_proj_w, c_proj_b, mask_self_attention):
    x = np.asarray(x)
    c_attn_w = np.asarray(c_attn_w)
    c_attn_b = np.asarray(c_attn_b)
    c_proj_w = np.asarray(c_proj_w)
    c_proj_b = np.asarray(c_proj_b)
    nc = get_graph()
    in_maps = _shard_inputs(x, c_attn_w, c_attn_b, c_proj_w, c_proj_b)
    res = run_bass_kernel_spmd(nc, in_maps, core_ids=list(range(8)))
    B = x.shape[0]
    out = np.empty((B, S, D), dtype=np.float32)
    for b in range(B):
        acc = res.results[2 * b]["outT"] + res.results[2 * b + 1]["outT"]
        out[b] = acc.T + c_proj_b[None, :].astype(np.float32)
    return out
